# revision 6
# baseline (speedup 1.0000x reference)
"""AWQ 4-bit quantized linear layer on 8 Trainium2 NeuronCores.

Computes out = x @ W.T + bias where W[o,i] = (q[o,i] - z[o,i//128]) * s[o,i//128],
q/z packed 8x int4 per int32.

Sharding: column-parallel (tensor-parallel on out_features). Each of the 8
cores gets qweight/qzeros/scales/bias rows [c*512, (c+1)*512) and the full
activation (shipped pre-transposed in bf16). Each core dequantizes its weight
shard on-chip (DVE nibble unpack + scale, PE transpose into [K, N] layout)
and runs a bf16 matmul with fp32 PSUM accumulation. Host concatenates the 8
[B, 512] outputs along the feature axis.
"""

import os
import sys

for _p in ("/opt/trn_rl_repo", "/root/.axon_site/_ro/trn_rl_repo"):
    if os.path.isdir(_p) and _p not in sys.path:
        sys.path.insert(0, _p)

import numpy as np
import ml_dtypes

import concourse.bass as bass
import concourse.tile as tile
from concourse import bacc, mybir
from concourse.masks import make_identity

# Full-problem shapes (hardcoded; harness contract)
B_FULL = 8192
I_FULL = 4096
O_FULL = 4096
N_CORES = 8
GROUP = 128
PACK = 8

BF16 = mybir.dt.bfloat16
F32 = mybir.dt.float32
I32 = mybir.dt.int32


def build_bass(B, I, OS, m_super=512):
    """Build the per-core SPMD program.

    B: batch rows, I: in_features, OS: out_features per core.
    m_super: batch columns processed per super-block (multiple of 128).
    """
    KT = I // 128          # k-tiles (contraction)
    OT = OS // 128         # o-part-tiles in the shard
    NP = I // PACK         # packed int32 words per row
    NG = I // GROUP        # quantization groups
    NGP = (NG + PACK - 1) // PACK
    MSn = B // m_super     # m super-blocks
    M4 = m_super // 128    # 128-row m-tiles per super-block

    nc = bacc.Bacc("TRN2", target_bir_lowering=False)

    xT_d = nc.dram_tensor("xT", [I, B], BF16, kind="ExternalInput")
    qw_d = nc.dram_tensor("qw", [OS, NP], I32, kind="ExternalInput")
    qz_d = nc.dram_tensor("qz", [OS, NGP], I32, kind="ExternalInput")
    sc_d = nc.dram_tensor("sc", [OS, NG], F32, kind="ExternalInput")
    bi_d = nc.dram_tensor("bi", [OS], F32, kind="ExternalInput")
    out_d = nc.dram_tensor("out", [B, OS], F32, kind="ExternalOutput")

    with tile.TileContext(nc) as tc:
        with (
            tc.tile_pool(name="const", bufs=1) as const,
            tc.tile_pool(name="wt", bufs=1) as wtp,
            tc.tile_pool(name="dq", bufs=2) as dq,
            tc.tile_pool(name="xp", bufs=2 * KT) as xp,
            tc.tile_pool(name="ob", bufs=4) as ob,
            tc.tile_pool(name="ps", bufs=8, space="PSUM") as ps,
        ):
            ident = const.tile([128, 128], BF16)
            make_identity(nc, ident[:])

            # bias broadcast to [128, OS] (varies along free dim of out tiles)
            bias_bc = const.tile([128, OS], F32)
            bi_ap = bi_d[:]
            nc.gpsimd.dma_start(
                out=bias_bc[:],
                in_=bass.AP(tensor=bi_ap.tensor, offset=0, ap=[[0, 128], [1, OS]]),
            )

            # Dequantized weight, [k-tile partition(i), KT, OS] bf16, resident
            WT = wtp.tile([128, KT, OS], BF16)

            # ---- dequantization ----
            for ot in range(OT):
                qw_t = dq.tile([128, NP], I32)
                nc.sync.dma_start(qw_t[:], qw_d[ot * 128:(ot + 1) * 128, :])
                s_t = dq.tile([128, NG], F32)
                nc.sync.dma_start(s_t[:], sc_d[ot * 128:(ot + 1) * 128, :])
                qz_t = dq.tile([128, NGP], I32)
                nc.sync.dma_start(qz_t[:], qz_d[ot * 128:(ot + 1) * 128, :])

                # unpack zero-points: z[o, g], g = 8*pc + j
                z_t = dq.tile([128, NG], I32)
                z_v = z_t.rearrange("p (pc j) -> p pc j", j=PACK)
                for j in range(PACK):
                    nc.vector.tensor_scalar(
                        out=z_v[:, :, j],
                        in0=qz_t[:],
                        scalar1=4 * j,
                        scalar2=0xF,
                        op0=mybir.AluOpType.logical_shift_right,
                        op1=mybir.AluOpType.bitwise_and,
                    )
                # int32 x f32 -> f32 (DVE converts inputs before the ALU)
                zs_t = dq.tile([128, NG], F32)
                nc.vector.tensor_mul(zs_t[:], z_t[:], s_t[:])

                # expand per-group scale / zero*scale along packed dim (16 words per group)
                s_full = dq.tile([128, NP], F32)
                zs_full = dq.tile([128, NP], F32)
                s_fv = s_full.rearrange("p (g r) -> p g r", r=16)
                zs_fv = zs_full.rearrange("p (g r) -> p g r", r=16)
                for r in range(16):
                    nc.vector.tensor_copy(s_fv[:, :, r], s_t[:])
                    nc.vector.tensor_copy(zs_fv[:, :, r], zs_t[:])

                # unpack + dequantize: W[o, 8p+j] = nib * s - z*s
                W_sb = dq.tile([128, I], BF16)
                W_v = W_sb.rearrange("p (pk j) -> p pk j", j=PACK)
                for j in range(PACK):
                    nib = dq.tile([128, NP], I32)
                    nc.vector.tensor_scalar(
                        out=nib[:],
                        in0=qw_t[:],
                        scalar1=4 * j,
                        scalar2=0xF,
                        op0=mybir.AluOpType.logical_shift_right,
                        op1=mybir.AluOpType.bitwise_and,
                    )
                    nibf = dq.tile([128, NP], F32)
                    nc.vector.tensor_mul(nibf[:], nib[:], s_full[:])
                    nc.vector.tensor_sub(W_v[:, :, j], nibf[:], zs_full[:])

                # transpose [128 o, 128 i] blocks -> WT[i, k, o]
                for k in range(KT):
                    tp = ps.tile([128, 128], BF16, name="acc", tag="acc")
                    nc.tensor.transpose(
                        tp[:], W_sb[:, k * 128:(k + 1) * 128], ident[:]
                    )
                    nc.vector.tensor_copy(
                        WT[:, k, ot * 128:(ot + 1) * 128], tp[:]
                    )

            # ---- matmul: out[b, o] = sum_i x[b, i] W[o, i] ----
            for ms in range(MSn):
                xs = []
                for k in range(KT):
                    xk = xp.tile([128, m_super], BF16)
                    nc.sync.dma_start(
                        xk[:],
                        xT_d[k * 128:(k + 1) * 128,
                             ms * m_super:(ms + 1) * m_super],
                    )
                    xs.append(xk)
                pss = []
                for m4 in range(M4):
                    pss.append(ps.tile([128, OS], F32, name="acc", tag="acc"))
                for k in range(KT):
                    for m4 in range(M4):
                        nc.tensor.matmul(
                            pss[m4][:],
                            xs[k][:, m4 * 128:(m4 + 1) * 128],
                            WT[:, k, :],
                            start=(k == 0),
                            stop=(k == KT - 1),
                        )
                for m4 in range(M4):
                    o_sb = ob.tile([128, OS], F32)
                    nc.vector.tensor_add(o_sb[:], pss[m4][:], bias_bc[:])
                    nc.sync.dma_start(
                        out_d[ms * m_super + m4 * 128:
                              ms * m_super + (m4 + 1) * 128, :],
                        o_sb[:],
                    )

    nc.compile()
    return nc


_NC_CACHE = {}


def _get_nc(B, I, OS):
    key = (B, I, OS)
    if key not in _NC_CACHE:
        _NC_CACHE[key] = build_bass(B, I, OS)
    return _NC_CACHE[key]


def make_in_maps(x, qweight, qzeros, scales, bias, n_cores=N_CORES):
    O = qweight.shape[0]
    OS = O // n_cores
    xT = np.ascontiguousarray(x.T).astype(ml_dtypes.bfloat16)
    in_maps = []
    for c in range(n_cores):
        sl = slice(c * OS, (c + 1) * OS)
        in_maps.append({
            "xT": xT,
            "qw": np.ascontiguousarray(qweight[sl]),
            "qz": np.ascontiguousarray(qzeros[sl]),
            "sc": np.ascontiguousarray(scales[sl]),
            "bi": np.ascontiguousarray(bias[sl]),
        })
    return in_maps


def kernel(x, qweight, qzeros, scales, bias):
    from concourse.bass_utils import run_bass_kernel_spmd

    B, I = x.shape
    O = qweight.shape[0]
    OS = O // N_CORES
    nc = _get_nc(B, I, OS)
    in_maps = make_in_maps(x, qweight, qzeros, scales, bias)
    res = run_bass_kernel_spmd(nc, in_maps, core_ids=list(range(N_CORES)))
    out = np.concatenate([res.results[c]["out"] for c in range(N_CORES)], axis=1)
    return out.astype(np.float32)
